# revision 14
# baseline (speedup 1.0000x reference)
# GCN layer kernel for Trainium2: out[b] = relu((a[b] @ x[b]) @ W) * mask[b]
#
# Sharding: data-parallel over the batch (graph) dim. B=8 graphs, 8 cores,
# one graph per core; W replicated. Inputs are the FULL tensors; shards are
# sliced host-side and the per-core outputs stacked back together.
#
# Per-core dataflow (a: [2048,2048], x: [2048,512], W: [512,512]):
#   - All matmuls run in bf16: 1 cycle/row on the PE at full clock (same
#     rate as f32r) and 1 cycle/row for PE transposes (fp32 pays 2).
#     rel-err ~3e-3 vs the fp32 reference; tolerance is 2e-2.
#   - a loads as fp32 [128,2048] strips (2 per chunk on each HWDGE queue),
#     DVE casts them to bf16, and the PE transposes them through PSUM in
#     quads of [128,128] tiles (4 per PSUM bank, DVE/ACT copy back into
#     at[p=m%128, mtile, ni, r=n%128]). A bf16 tile transpose is 128
#     cycles, so all 256 tiles cost ~13.6us of PE time -- the DMA fabric
#     (~420 GB/s/core, shared by all queues; measured) carries only the
#     25MB of real input/output traffic.
#       mm1: tT[f,n] = sum_m x[m,f] * aT[m,n]   (lhsT = x, rhs = aT)
#       mm2: out[n,d] = sum_f tT[f,n] * W[f,d]  (lhsT = tT, rhs = W)
#   - HAM clock gate (measured): PE activity is sampled in 3.41us epochs;
#     an epoch with a multi-us idle gap drops the clock to k=4/8, making
#     every matmul 379ns instead of 213ns. The schedule keeps the PE
#     stream gapless: warm-up matmuls cover the initial DMA window,
#     transposes of chunk j+1 interleave into chunk j's mm1 stream (one
#     quad per 4 matmuls), and mm2 runs a full chunk behind mm1 so its
#     inputs are always long since landed.
#   - x loads fp32 in two column-halves (mm1 is fi-outer, so fi=0,1 only
#     need the first half); DVE casts into the resident bf16 tile.
#     mask[n] = any(x[n,:] != 0) via |x| row-sums (ACT), applied as the
#     scale input of the fused ReLU.
#
# Queues: sync = a strips ni=0,1 + w; scalar = a strips ni=2,3 + x halves;
#   gpsimd = output stores. PSUM: 4 mm1 banks + 2 transpose banks + 2 mm2
#   banks (mm2 runs in two ns-pair waves; warms borrow the mm2 pool).

import numpy as np

B, N, F, D = 8, 2048, 512, 512
P = 128
NT = N // P        # 16 row-tiles of n (and of m, since a is square)
FT = F // P        # 4 tiles of f
NCHUNK = 512       # n is processed in chunks of 512 rows
NJ = N // NCHUNK   # 4
NSUB = NCHUNK // P # 4

_CACHE = {}


def _build_nc():
    from contextlib import ExitStack

    from concourse import bacc, mybir, tile
    from concourse.masks import make_identity

    f32 = mybir.dt.float32
    bf16 = mybir.dt.bfloat16
    AF = mybir.ActivationFunctionType

    nc = bacc.Bacc(None)
    a_d = nc.dram_tensor("a", [N, N], f32, kind="ExternalInput")
    x_d = nc.dram_tensor("x", [N, F], f32, kind="ExternalInput")
    w_d = nc.dram_tensor("kernel", [F, D], f32, kind="ExternalInput")
    o_d = nc.dram_tensor("out", [N, D], f32, kind="ExternalOutput")

    with tile.TileContext(nc) as tc, ExitStack() as ctx:
        const = ctx.enter_context(tc.tile_pool(name="const", bufs=1))
        xp = ctx.enter_context(tc.tile_pool(name="xp", bufs=1))
        wp = ctx.enter_context(tc.tile_pool(name="wp", bufs=1))
        xs = ctx.enter_context(tc.tile_pool(name="xs", bufs=2))
        afp = ctx.enter_context(tc.tile_pool(name="afp", bufs=4))
        abp = ctx.enter_context(tc.tile_pool(name="abp", bufs=8))
        atp = ctx.enter_context(tc.tile_pool(name="atp", bufs=2))
        ttp = ctx.enter_context(tc.tile_pool(name="ttp", bufs=2))
        outp = ctx.enter_context(tc.tile_pool(name="outp", bufs=4))
        scr = ctx.enter_context(tc.tile_pool(name="scr", bufs=2))
        ps_mm = ctx.enter_context(tc.tile_pool(name="ps_mm", bufs=4, space="PSUM"))
        ps_tp = ctx.enter_context(tc.tile_pool(name="ps_tp", bufs=2, space="PSUM"))
        ps_o = ctx.enter_context(tc.tile_pool(name="ps_o", bufs=2, space="PSUM"))

        ident = const.tile([P, P], f32)
        make_identity(nc, ident[:])
        ident_b = const.tile([P, P], bf16)
        nc.vector.tensor_copy(ident_b[:], ident[:])

        def warm_fp32():
            # fp32 identity matmul: counts as HAM activity, output unused.
            pw = ps_o.tile([P, D], f32, tag="pso", name="pw")
            nc.tensor.matmul(
                pw[:, :P], lhsT=ident[:], rhs=ident[:], start=True, stop=True
            )

        def warm_bf16(lhs, rhs):
            # bf16 warm matmul on freshly-cast tiles: fires as the cast
            # lands, pacing PE activity through the DMA wait.
            pw = ps_o.tile([P, D], f32, tag="pso", name="pwb")
            nc.tensor.matmul(
                pw[:, : rhs.shape[-1]], lhsT=lhs, rhs=rhs, start=True, stop=True
            )

        for _ in range(8):
            warm_fp32()

        x_b = xp.tile([P, NT, F], bf16)
        w_b = wp.tile([P, FT, D], bf16)
        sumabs = const.tile([P, NT], f32)
        mask_sb = const.tile([P, NT], f32)

        ab_strips = [[None] * NSUB for _ in range(NJ)]
        at_tiles = [None] * NJ
        cb = 0  # copyback counter for DVE/ACT alternation

        def copyback(dst, src):
            nonlocal cb
            if cb % 2 == 0:
                nc.scalar.copy(dst, src)
            else:
                nc.vector.tensor_copy(dst, src)
            cb += 1

        def load_strip(nj, ni):
            queue = nc.sync if ni < 2 else nc.scalar
            af = afp.tile([P, N], f32, tag="af", name="af")
            r0 = (nj * NSUB + ni) * P
            queue.dma_start(af[:], a_d[r0 : r0 + P, :])
            return af

        def cast_strip(nj, ni, af, warm=False):
            ab = abp.tile([P, N], bf16, tag="ab", name="ab")
            nc.vector.tensor_copy(ab[:], af[:])
            ab_strips[nj][ni] = ab
            if warm:
                warm_bf16(ab[:, 0:P], ab[:, 0:NCHUNK])
                warm_bf16(ab[:, P : 2 * P], ab[:, NCHUNK : 2 * NCHUNK])

        def t_quad(nj, slot):
            # PE-transpose 4 tiles (strip ni, m-tiles q*4..q*4+3) through one
            # PSUM bank, then DVE/ACT copy into at[p, mtile, ni, r].
            ni, q = divmod(slot, 4)
            if at_tiles[nj] is None:
                at_tiles[nj] = atp.tile([P, NT, NSUB, P], bf16, tag="at", name="at")
            ps = ps_tp.tile([P, NCHUNK], bf16, tag="pst", name="pst")
            ab = ab_strips[nj][ni]
            for k in range(4):
                mi = q * 4 + k
                nc.tensor.transpose(
                    ps[:, k * P : (k + 1) * P], ab[:, mi * P : (mi + 1) * P],
                    ident_b[:],
                )
            copyback(
                at_tiles[nj][:, q * 4 : (q + 1) * 4, ni, :],
                ps[:].rearrange("p (a f) -> p a f", a=4),
            )

        # ---- preamble ----
        # load priority: the DMA fabric only sustains ~200-300 GB/s in the
        # first ~25us, so chunk 0 + the first x half go first on both
        # queues; chunk 1 and the rest of x follow; w rides the idle
        # gpsimd queue as a cast-DMA (fp32->bf16 in flight).
        af00 = load_strip(0, 0)
        af02 = load_strip(0, 2)
        af01 = load_strip(0, 1)
        af03 = load_strip(0, 3)
        xlA = xs.tile([P, NT, F // 2], f32, tag="xl", name="xlA")
        nc.sync.dma_start(
            xlA[:], x_d[:, 0 : F // 2].rearrange("(o p) f -> p o f", p=P)
        )
        xlB = xs.tile([P, NT, F // 2], f32, tag="xl", name="xlB")
        nc.scalar.dma_start(
            xlB[:], x_d[:, F // 2 : F].rearrange("(o p) f -> p o f", p=P)
        )
        nc.gpsimd.dma_start(w_b[:], w_d[:].rearrange("(o p) d -> p o d", p=P))
        af1 = [load_strip(1, ni) for ni in range(NSUB)]

        cast_strip(0, 0, af00, warm=True)
        cast_strip(0, 2, af02, warm=True)
        for slot in (0, 1, 2, 3):       # strip 0 quads
            t_quad(0, slot)
        warm_fp32()
        for slot in (8, 9, 10, 11):     # strip 2 quads
            t_quad(0, slot)
        cast_strip(0, 1, af01, warm=True)
        warm_fp32()
        for slot in (4, 5, 6, 7):       # strip 1 quads
            t_quad(0, slot)
        cast_strip(0, 3, af03, warm=True)
        warm_fp32()
        for slot in (12, 13, 14, 15):   # strip 3 quads
            t_quad(0, slot)
        nc.vector.tensor_copy(x_b[:, :, 0 : F // 2], xlA[:])
        warm_bf16(x_b[:, 0, 0:P], x_b[:, 0, 0 : F // 2])
        warm_bf16(x_b[:, 1, 0:P], x_b[:, 1, 0 : F // 2])
        nc.vector.tensor_copy(x_b[:, :, F // 2 : F], xlB[:])
        warm_bf16(x_b[:, 2, 0:P], x_b[:, 2, :])
        cast_strip(1, 0, af1[0], warm=True)
        cast_strip(1, 1, af1[1], warm=True)
        cast_strip(1, 2, af1[2], warm=True)
        cast_strip(1, 3, af1[3], warm=True)

        # mask reductions once x_b is assembled (ACT is idle until the
        # first ReLU at ~35us)
        for o in range(NT):
            abs_scr = scr.tile([P, F], bf16, tag="abs_scr")
            nc.scalar.activation(
                abs_scr[:], x_b[:, o, :], AF.Abs, accum_out=sumabs[:, o : o + 1]
            )
        nc.vector.tensor_scalar(
            mask_sb[:], sumabs[:], 0.0, None, mybir.AluOpType.is_gt
        )

        # ---- main loop ----
        tts = [None] * NJ
        po_banks = {}

        def mm2_wave(nj, w):
            # half of mm2 for chunk nj: output tiles ns = 2w, 2w+1
            # accumulated over all fi in 2 PSUM banks, then fused
            # relu(mask * po) -> SBUF -> store via the gpsimd queue.
            tt = tts[nj]
            po_banks[nj] = pos = [
                ps_o.tile([P, D], f32, tag="pso", name=f"po_{nj}_{w}_{i}")
                for i in range(2)
            ]
            for fi in range(FT):
                for i in range(2):
                    ns = 2 * w + i
                    nc.tensor.matmul(
                        pos[i][:],
                        lhsT=tt[:, fi, ns * P : (ns + 1) * P],
                        rhs=w_b[:, fi],
                        start=(fi == 0),
                        stop=(fi == FT - 1),
                    )
            for i in range(2):
                ns = 2 * w + i
                ni = nj * NSUB + ns
                ob = outp.tile([P, D], f32, tag="ob")
                nc.scalar.activation(
                    ob[:], pos[i][:], AF.Relu, scale=mask_sb[:, ni : ni + 1]
                )
                nc.gpsimd.dma_start(o_d[ni * P : (ni + 1) * P, :], ob[:])

        for nj in range(NJ):
            # loads for chunk nj+2 (queue-split 2+2) and casts
            if nj + 2 < NJ:
                af_n = [load_strip(nj + 2, ni) for ni in range(NSUB)]
                for ni in range(NSUB):
                    cast_strip(nj + 2, ni, af_n[ni])

            at_cur = at_tiles[nj]
            tt = ttp.tile([P, FT, NCHUNK], bf16, tag="tt")
            tts[nj] = tt
            pt = [
                ps_mm.tile([P, NCHUNK], f32, tag="psm", name=f"pt_{nj}_{fi}")
                for fi in range(FT)
            ]
            slots = iter(range(16))
            for fi in range(FT):
                for mi in range(NT):
                    nc.tensor.matmul(
                        pt[fi][:],
                        lhsT=x_b[:, mi, fi * P : (fi + 1) * P],
                        rhs=at_cur[:, mi, :, :],
                        start=(mi == 0),
                        stop=(mi == NT - 1),
                    )
                    # transposes of chunk nj+1 ride the mm1 stream, one
                    # quad per 4 matmuls
                    if mi % 4 == 3 and nj + 1 < NJ:
                        t_quad(nj + 1, next(slots))
                copyback(tt[:, fi], pt[fi][:])
                # mm2 of the previous chunk, two ns-wave groups per chunk:
                # inputs landed a full chunk ago, so the PE never waits.
                if nj > 0:
                    if fi == 1:
                        mm2_wave(nj - 1, 0)
                    elif fi == 3:
                        mm2_wave(nj - 1, 1)

        mm2_wave(NJ - 1, 0)
        mm2_wave(NJ - 1, 1)

    nc.compile()
    return nc


def get_nc():
    if "nc" not in _CACHE:
        _CACHE["nc"] = _build_nc()
    return _CACHE["nc"]


def kernel(**inputs) -> np.ndarray:
    from concourse.bass_utils import run_bass_kernel_spmd

    x = np.ascontiguousarray(np.asarray(inputs["x"], dtype=np.float32))
    a = np.ascontiguousarray(np.asarray(inputs["a"], dtype=np.float32))
    w = np.ascontiguousarray(np.asarray(inputs["kernel"], dtype=np.float32))
    assert x.shape == (B, N, F) and a.shape == (B, N, N) and w.shape == (F, D)

    nc = get_nc()
    in_maps = [{"a": a[b], "x": x[b], "kernel": w} for b in range(B)]
    res = run_bass_kernel_spmd(nc, in_maps, core_ids=list(range(B)))
    return np.stack([res.results[b]["out"] for b in range(B)], axis=0)
